# revision 15
# baseline (speedup 1.0000x reference)
"""Tensor-parallel dense transformer (4-layer, D=1024, H=16, F=4096, S=2048,
V=32000 tied lm_head) on 8 Trainium2 NeuronCores via Bass/Tile.

v3 over v2:
  - RoPE via DVE stream_shuffle on pair-interleaved head rows (host permutes
    Wq/Wk columns + cos/sin tables) -> drops the 2 extra permuted projections.
  - Attention o_proj restructured as AllGather(o_heads) -> D-sharded full
    o_proj -> AllGather(delta) -> local residual add. Replaces 4 AllReduces
    per layer with 8 cheap AllGathers (128KB/rank each).
  - Two-head interleaved attention inner loop: the two local heads' score
    matmuls go to different PE row-groups (K=64) and run concurrently.
  - RMSNorm 1/rms via Ln+Exp (same ACT table set as attention exp); fp16
    broadcast operand for 2x DVE mode.
  - FFN phase in 2 halves (AllReduce of 2MB each instead of 4x1MB), fewer
    ACT table swaps (exp-set / silu-set alternation per half, not per chunk).

Sharding (Megatron TP over 8 cores):
  - QKV: output dim (heads) sharded -> 2 heads/core (EL=128 cols)
  - o_proj: AllGather head outputs, each core computes a D/8 output shard
    over the full contraction, AllGather shards -> local residual add
  - gate/up: F sharded -> FL=512 cols/core; down: partial sums AllReduced
  - lm_head: vocab sharded -> VL=4000 logits/core, host concat

kernel(**inputs) takes the FULL unsharded inputs (as reference.setup_inputs)
and returns full logits [B, S, V] fp32.
"""
import sys
sys.path.insert(0, "/opt/trn_rl_repo")

import numpy as np
import ml_dtypes
from contextlib import ExitStack

import concourse.bass as bass
import concourse.mybir as mybir
import concourse.tile as tile
from concourse import bacc
from concourse.bass import ts

BF = np.float16
F32 = mybir.dt.float32
BF16 = mybir.dt.float16
AF = mybir.ActivationFunctionType
ALU = mybir.AluOpType

V, D, H, F, L, S, B = 32000, 1024, 16, 4096, 4, 2048, 1
NC_CORES = 8
ROPE_BASE = 10000.0
EPS = 1e-6

HD = 64
HL = H // NC_CORES          # heads per core (2)
EL = HL * HD                # local qkv width (128)
FL = F // NC_CORES          # local ffn width (512)
VL = V // NC_CORES          # local vocab (4000)
NT = S // 128               # 128-col s-tiles (16)
NSC = S // 512              # 512-col s-chunks (4)
ND = D // 128               # d-chunks (8)
NFT = FL // 128             # f-tiles (4)

NCH = 4                     # s-chunks in attention pipeline
SW = S // NCH               # 512
NFH = 2                     # FFN halves
SWF = S // NFH              # 1024
SCF = SWF // 512            # 512-chunks per ffn half (2)

# stream_shuffle mask: swap adjacent partitions within each 32-quadrant
SWAP_MASK = [i ^ 1 for i in range(32)]


def _dims():
    return HD, HL, EL, FL, VL, NT, NSC, ND, NFT


def build_nc():
    nc = bacc.Bacc("TRN2", target_bir_lowering=False, debug=False,
                   num_devices=NC_CORES)

    hid_ext = nc.dram_tensor("hid0T", [D, S], BF16, kind="ExternalInput")
    wq_ext = nc.dram_tensor("wqT", [L, D, EL], BF16, kind="ExternalInput")
    wk_ext = nc.dram_tensor("wkT", [L, D, EL], BF16, kind="ExternalInput")
    wv_ext = nc.dram_tensor("wvT", [L, D, EL], BF16, kind="ExternalInput")
    wo_ext = nc.dram_tensor("woT", [L, D, 128], BF16, kind="ExternalInput")
    wg_ext = nc.dram_tensor("wgT", [L, D, FL], BF16, kind="ExternalInput")
    wu_ext = nc.dram_tensor("wuT", [L, D, FL], BF16, kind="ExternalInput")
    wd_ext = nc.dram_tensor("wdT", [L, FL, D], BF16, kind="ExternalInput")
    embT_ext = nc.dram_tensor("embT", [D, VL], BF16, kind="ExternalInput")
    cos_ext = nc.dram_tensor("cosT", [EL, S], BF16, kind="ExternalInput")
    sin_ext = nc.dram_tensor("sinT", [EL, S], BF16, kind="ExternalInput")
    mask_ext = nc.dram_tensor("maskT", [4, 128, 512], BF16, kind="ExternalInput")
    logits_ext = nc.dram_tensor("logits", [S, VL], BF16, kind="ExternalOutput")

    cc_o_in = nc.dram_tensor("cc_o_in", [NCH, 128, SW], BF16)
    cc_o_out = nc.dram_tensor("cc_o_out", [NCH, D, SW], BF16, addr_space="Shared")
    cc_d_in = nc.dram_tensor("cc_d_in", [NCH, 128, SW], BF16)
    cc_d_out = nc.dram_tensor("cc_d_out", [NCH, D, SW], BF16, addr_space="Shared")
    cc_f_in = nc.dram_tensor("cc_f_in", [NFH, D, SWF], BF16)
    cc_f_out = nc.dram_tensor("cc_f_out", [NFH, D, SWF], BF16, addr_space="Shared")
    RG = [list(range(NC_CORES))]

    with tile.TileContext(nc) as tc, ExitStack() as ctx:
        const_p = ctx.enter_context(tc.tile_pool(name="const", bufs=1))
        persist_p = ctx.enter_context(tc.tile_pool(name="persist", bufs=1))
        work_p = ctx.enter_context(tc.tile_pool(name="work", bufs=2))

        hT = persist_p.tile([128, ND, S], BF16)     # residual stream, d-major
        nc.sync.dma_start(hT[:], hid_ext[:, :].rearrange("(c p) s -> p c s", p=128))
        xT = persist_p.tile([128, ND, S], BF16)     # normed input, d-major

        cos_sb = const_p.tile([EL, S], BF16)
        nc.sync.dma_start(cos_sb[:], cos_ext[:, :])
        sin_sb = const_p.tile([EL, S], BF16)
        nc.sync.dma_start(sin_sb[:], sin_ext[:, :])
        mask_sb = const_p.tile([128, 4, 512], BF16)
        nc.sync.dma_start(mask_sb[:], mask_ext[:, :, :].rearrange("i p b -> p i b"))
        ones_sb = const_p.tile([128, 1], BF16)
        nc.gpsimd.memset(ones_sb[:], 1.0)
        eps1 = const_p.tile([1, 1], F32)
        nc.gpsimd.memset(eps1[:], EPS)

        VQ = VL // 4
        emb_p = ctx.enter_context(tc.tile_pool(name="embp", bufs=1))
        emb_tiles = {}

        def norm_chunk(g):
            """xT[:, :, g*512 chunk] = hT / rms(hT) over those s-columns."""
            gsl = ts(g, SW)
            with tc.tile_pool(name="nps", bufs=1, space="PSUM") as nps:
                ssq = nps.tile([1, SW], F32, tag="ssq")
                sqs = []
                for dc in range(ND):
                    sq = work_p.tile([128, SW], BF16, tag="sq", bufs=2)
                    nc.scalar.activation(sq[:], hT[:, dc, gsl], AF.Square)
                    sqs.append(sq)
                for dc in range(ND):
                    nc.tensor.matmul(ssq[0:1, :], ones_sb[:], sqs[dc][:],
                                     start=(dc == 0), stop=(dc == ND - 1))
                lns = work_p.tile([1, SW], F32, tag="lns", bufs=1)
                nc.scalar.activation(lns[:], ssq[:], AF.Ln, scale=1.0 / D,
                                     bias=eps1[:])
                inv = work_p.tile([1, SW], BF16, tag="inv", bufs=1)
                nc.scalar.activation(inv[:], lns[:], AF.Exp, scale=-0.5)
                binv = work_p.tile([128, SW], BF16, tag="binv", bufs=2)
                nc.gpsimd.partition_broadcast(binv[:], inv[:], channels=128)
                for dc in range(ND):
                    nc.vector.tensor_tensor(xT[:, dc, gsl], hT[:, dc, gsl],
                                            binv[:], ALU.mult)

        # initial norm (layer-0 attn input; attn_norm_w folded into Wq/Wk/Wv)
        for g in range(NCH):
            norm_chunk(g)

        with ExitStack() as lctx:
            loop_p = lctx.enter_context(tc.tile_pool(name="loop", bufs=1))
            w_p = lctx.enter_context(tc.tile_pool(name="wts", bufs=1))
            ps = lctx.enter_context(tc.tile_pool(name="ps", bufs=1, space="PSUM"))

            qsb = loop_p.tile([EL, S], BF16)
            ksb = loop_p.tile([EL, S], BF16)
            o_in = loop_p.tile([EL, S], BF16)
            v_store = loop_p.tile([128, NT, HL, 65], BF16)
            nc.gpsimd.memset(v_store[:, :, :, 64:65], 1.0)

            def rope_apply(src_ps, dst, g):
                # src_ps: [128, SW] fp32 psum; dst columns of chunk g
                sl = ts(g, SW)
                qsh = work_p.tile([128, SW], F32, tag="qsh", bufs=2)
                nc.vector.stream_shuffle(qsh[:], src_ps[:], SWAP_MASK)
                tq = work_p.tile([128, SW], BF16, tag="ropet", bufs=2)
                nc.vector.tensor_tensor(tq[:], src_ps[:], cos_sb[:, sl], ALU.mult)
                u = work_p.tile([128, SW], BF16, tag="ropeu", bufs=2)
                nc.vector.tensor_tensor(u[:], qsh[:], sin_sb[:, sl], ALU.mult)
                nc.vector.tensor_tensor(dst[:, sl], tq[:], u[:], ALU.add)

            def qkv_chunk(g, wq_sb, wk_sb, wv_sb):
                gsl = ts(g, SW)
                qps = ps.tile([128, SW], F32, tag="qk", bufs=2)
                for dc in range(ND):
                    nc.tensor.matmul(qps[:], wq_sb[:, dc, :], xT[:, dc, gsl],
                                     start=(dc == 0), stop=(dc == ND - 1))
                rope_apply(qps, qsb, g)
                kps = ps.tile([128, SW], F32, tag="qk", bufs=2)
                for dc in range(ND):
                    nc.tensor.matmul(kps[:], wk_sb[:, dc, :], xT[:, dc, gsl],
                                     start=(dc == 0), stop=(dc == ND - 1))
                rope_apply(kps, ksb, g)
                vps = ps.tile([128, 4, HL, 64], F32, tag="vv", bufs=1)
                for tt in range(4):
                    t = 4 * g + tt
                    for dc in range(ND):
                        nc.tensor.matmul(vps[:, tt, :, :],
                                         xT[:, dc, ts(t, 128)], wv_sb[:, dc, :],
                                         start=(dc == 0), stop=(dc == ND - 1))
                for tt in range(4):
                    nc.vector.tensor_copy(v_store[:, 4 * g + tt, :, 0:64],
                                          vps[:, tt, :, :])

            def attn_chunk(j):
                """Attention for q-chunk j (512 q cols), both heads interleaved."""
                nkc = 4 * (j + 1)           # 128-key blocks covered
                avp = [ps.tile([65, 512], F32, tag=f"av{h}", bufs=1,
                               name=f"avp{h}")
                       for h in range(HL)]
                for kc in range(nkc):
                    psbs = []
                    for h in range(HL):
                        hb = 64 * h
                        scp = ps.tile([128, 512], F32, tag="sc", bufs=2)
                        nc.tensor.matmul(scp[:], ksb[hb:hb + 64, ts(kc, 128)],
                                         qsb[hb:hb + 64, ts(j, 512)],
                                         start=True, stop=True)
                        psb = work_p.tile([128, 512], BF16, tag="p", bufs=4)
                        nc.scalar.activation(psb[:], scp[:], AF.Exp, scale=0.125)
                        if kc >= 4 * j:     # diagonal block: causal mask
                            nc.vector.tensor_tensor(
                                psb[:], psb[:], mask_sb[:, kc - 4 * j, :],
                                ALU.mult)
                        psbs.append(psb)
                    for h in range(HL):
                        nc.tensor.matmul(avp[h][:], v_store[:, kc, h, :],
                                         psbs[h][:],
                                         start=(kc == 0), stop=(kc == nkc - 1))
                for h in range(HL):
                    hb = 64 * h
                    srow = work_p.tile([1, 512], F32, tag="srow", bufs=2)
                    nc.vector.tensor_copy(srow[:], avp[h][64:65, :])
                    srec = work_p.tile([1, 512], F32, tag="srec", bufs=2)
                    nc.vector.reciprocal_approx_fast(srec[:], srow[:])
                    bcsb = work_p.tile([64, 512], F32, tag="bcsb", bufs=2)
                    nc.gpsimd.partition_broadcast(bcsb[:], srec[:], channels=64)
                    nc.vector.tensor_tensor(o_in[hb:hb + 64, ts(j, 512)],
                                            avp[h][0:64, :], bcsb[:], ALU.mult)
                nc.sync.dma_start(cc_o_in[j], o_in[:, ts(j, 512)])
                nc.gpsimd.collective_compute(
                    "AllGather", ALU.bypass, replica_groups=RG,
                    ins=[cc_o_in[j].opt()], outs=[cc_o_out[j].opt()])

            def oproj_chunk(g, wo_sb):
                ofull = work_p.tile([128, ND, SW], BF16, tag="big8", bufs=1)
                nc.gpsimd.dma_start(
                    ofull[:], cc_o_out[g].rearrange("(c p) s -> p c s", p=128))
                dps = ps.tile([128, SW], F32, tag="vv", bufs=1)
                for dc in range(ND):
                    nc.tensor.matmul(dps[:], wo_sb[:, dc, :], ofull[:, dc, :],
                                     start=(dc == 0), stop=(dc == ND - 1))
                dsb = work_p.tile([128, SW], BF16, tag="dsb", bufs=2)
                nc.vector.tensor_copy(dsb[:], dps[:])
                nc.sync.dma_start(cc_d_in[g], dsb[:])
                nc.gpsimd.collective_compute(
                    "AllGather", ALU.bypass, replica_groups=RG,
                    ins=[cc_d_in[g].opt()], outs=[cc_d_out[g].opt()])

            def ffn_half(hf, wg_sb, wu_sb, wd_sb):
                # residual add of attention deltas for the two 512-chunks
                for g in (2 * hf, 2 * hf + 1):
                    delta = work_p.tile([128, ND, SW], BF16, tag="big8", bufs=1)
                    nc.gpsimd.dma_start(
                        delta[:],
                        cc_d_out[g].rearrange("(c p) s -> p c s", p=128))
                    nc.vector.tensor_tensor(hT[:, :, ts(g, SW)],
                                            hT[:, :, ts(g, SW)], delta[:],
                                            ALU.add)
                    norm_chunk(g)
                # gate/up/down over the 1024-col half
                gsc = work_p.tile([128, NFT, SWF], BF16, tag="gsc", bufs=1)
                for ft in range(NFT):
                    for scc in range(SCF):
                        gps = ps.tile([128, 512], F32, tag="qk", bufs=2)
                        for dc in range(ND):
                            nc.tensor.matmul(gps[:],
                                             wg_sb[:, dc, ts(ft, 128)],
                                             xT[:, dc, ts(2 * hf + scc, 512)],
                                             start=(dc == 0), stop=(dc == ND - 1))
                        sg = work_p.tile([128, 512], BF16, tag="sg", bufs=2)
                        nc.scalar.activation(sg[:], gps[:], AF.Silu)
                        ups = ps.tile([128, 512], F32, tag="qk", bufs=2)
                        for dc in range(ND):
                            nc.tensor.matmul(ups[:],
                                             wu_sb[:, dc, ts(ft, 128)],
                                             xT[:, dc, ts(2 * hf + scc, 512)],
                                             start=(dc == 0), stop=(dc == ND - 1))
                        nc.vector.tensor_tensor(gsc[:, ft, ts(scc, 512)],
                                                ups[:], sg[:], ALU.mult)
                for scc in range(SCF):
                    par = work_p.tile([128, ND, 512], BF16, tag="par", bufs=1)
                    for et in range(ND):
                        dps = ps.tile([128, 512], F32, tag="sc", bufs=2)
                        for fc in range(NFT):
                            nc.tensor.matmul(dps[:], wd_sb[:, fc, ts(et, 128)],
                                             gsc[:, fc, ts(scc, 512)],
                                             start=(fc == 0), stop=(fc == NFT - 1))
                        nc.vector.scalar_tensor_tensor(
                            par[:, et, :],
                            hT[:, et, ts(2 * hf + scc, 512)],
                            1.0 / NC_CORES, dps[:], ALU.mult, ALU.add)
                    nc.sync.dma_start(
                        cc_f_in[hf][:, ts(scc, 512)]
                        .rearrange("(c p) s -> p c s", p=128), par[:])
                nc.gpsimd.collective_compute(
                    "AllReduce", ALU.add, replica_groups=RG,
                    ins=[cc_f_in[hf].opt()], outs=[cc_f_out[hf].opt()])

            for l in range(L):
                if l == 1:
                    emb0_sb = emb_p.tile([128, ND, VQ], BF16, tag="emb")
                    nc.sync.dma_start(
                        emb0_sb[:],
                        embT_ext[:, 0:VQ].rearrange("(c p) v -> p c v", p=128))
                    emb_tiles[0] = emb0_sb
                wq_sb = w_p.tile([128, ND, EL], BF16, tag="wq")
                nc.sync.dma_start(wq_sb[:], wq_ext[l].rearrange("(c p) e -> p c e", p=128))
                wk_sb = w_p.tile([128, ND, EL], BF16, tag="wk")
                nc.sync.dma_start(wk_sb[:], wk_ext[l].rearrange("(c p) e -> p c e", p=128))
                wv_sb = w_p.tile([128, ND, EL], BF16, tag="wv")
                nc.sync.dma_start(wv_sb[:], wv_ext[l].rearrange("(c p) e -> p c e", p=128))
                wo_sb = w_p.tile([128, ND, 128], BF16, tag="wo")
                nc.sync.dma_start(wo_sb[:], wo_ext[l].rearrange("(c p) e -> p c e", p=128))
                wg_sb = w_p.tile([128, ND, FL], BF16, tag="wg")
                nc.sync.dma_start(wg_sb[:], wg_ext[l].rearrange("(c p) f -> p c f", p=128))
                wu_sb = w_p.tile([128, ND, FL], BF16, tag="wu")
                nc.sync.dma_start(wu_sb[:], wu_ext[l].rearrange("(c p) f -> p c f", p=128))
                wd_sb = w_p.tile([128, NFT, D], BF16, tag="wd")
                nc.sync.dma_start(wd_sb[:], wd_ext[l].rearrange("(c p) e -> p c e", p=128))

                # ---- phase A/B interleaved: norm+QKV+attention per chunk,
                #      o_proj for chunk g-1 pipelined behind its AllGather ----
                for g in range(NCH):
                    if l > 0:
                        hf, sc0 = g // 2, (g % 2) * 512
                        nc.gpsimd.dma_start(
                            hT[:, :, ts(g, SW)],
                            cc_f_out[hf][:, sc0:sc0 + 512]
                            .rearrange("(c p) s -> p c s", p=128))
                        norm_chunk(g)
                    qkv_chunk(g, wq_sb, wk_sb, wv_sb)
                    attn_chunk(g)
                    if g >= 1:
                        oproj_chunk(g - 1, wo_sb)
                oproj_chunk(NCH - 1, wo_sb)

                # ---- phase C: FFN halves ----
                for hf in range(NFH):
                    ffn_half(hf, wg_sb, wu_sb, wd_sb)

        # ---- lm_head (final_norm_w folded into embT); vocab in quarters ----
        vchunks = [(0, 512), (512, VQ - 512)]
        TPG = NT // NCH

        def lm_t(lps, t, v0, emb_sb):
            lp = lps.tile([128, VQ], F32, tag="lm")
            for dc in range(ND):
                for (vv, vn) in vchunks:
                    nc.tensor.matmul(lp[:, vv:vv + vn],
                                     xT[:, dc, ts(t, 128)],
                                     emb_sb[:, dc, vv:vv + vn],
                                     start=(dc == 0), stop=(dc == ND - 1))
            lsb = work_p.tile([128, VQ], BF16, tag="lsb", bufs=2)
            nc.scalar.activation(lsb[:, 0:512], lp[:, 0:512], AF.Copy)
            nc.vector.tensor_copy(lsb[:, 512:VQ], lp[:, 512:VQ])
            nc.sync.dma_start(logits_ext[ts(t, 128), v0:v0 + VQ], lsb[:])

        for g in range(NCH):
            hf, sc0 = g // 2, (g % 2) * 512
            nc.gpsimd.dma_start(
                hT[:, :, ts(g, SW)],
                cc_f_out[hf][:, sc0:sc0 + 512]
                .rearrange("(c p) s -> p c s", p=128))
            norm_chunk(g)
            with tc.tile_pool(name="lmps", bufs=2, space="PSUM") as lps:
                for t in range(g * TPG, (g + 1) * TPG):
                    lm_t(lps, t, 0, emb_tiles[0])
        for q in range(1, 4):
            embq_sb = emb_p.tile([128, ND, VQ], BF16, tag="emb",
                                 name=f"embq{q}")
            nc.sync.dma_start(
                embq_sb[:],
                embT_ext[:, q * VQ:(q + 1) * VQ]
                .rearrange("(c p) v -> p c v", p=128))
            with tc.tile_pool(name="lmps", bufs=2, space="PSUM") as lps:
                for t in range(NT):
                    lm_t(lps, t, q * VQ, embq_sb)

    nc.compile()
    return nc


def host_prep(inputs):
    """Full inputs -> per-core in_maps (list of dicts of np arrays)."""
    emb = np.ascontiguousarray(np.asarray(inputs["emb"], np.float32))
    ids = np.asarray(inputs["input_ids"]).reshape(-1)
    hid0T = np.ascontiguousarray(emb[ids].T).astype(BF)   # [D, S]

    anw = np.asarray(inputs["attn_norm_w"], np.float32)
    fnw = np.asarray(inputs["ffn_norm_w"], np.float32)
    finw = np.asarray(inputs["final_norm_w"], np.float32)
    Wq = np.asarray(inputs["Wq"], np.float32)
    Wk = np.asarray(inputs["Wk"], np.float32)
    Wv = np.asarray(inputs["Wv"], np.float32)
    Wo = np.asarray(inputs["Wo"], np.float32)
    Wg = np.asarray(inputs["Wg"], np.float32)
    Wu = np.asarray(inputs["Wu"], np.float32)
    Wd = np.asarray(inputs["Wd"], np.float32)

    # pair-interleaved row order within each head: partners adjacent
    pair_perm = np.empty(HD, dtype=int)
    pair_perm[0::2] = np.arange(HD // 2)
    pair_perm[1::2] = np.arange(HD // 2, HD)
    full_perm = np.concatenate([h * HD + pair_perm for h in range(HL)])

    # rope tables [HD, S] in pair-interleaved order
    inv_freq = 1.0 / (ROPE_BASE ** (np.arange(0, HD, 2, dtype=np.float32) / HD))
    ang = np.arange(S, dtype=np.float32)[:, None] * inv_freq[None, :]   # [S, HD/2]
    ang = np.concatenate([ang, ang], axis=1)                            # [S, HD]
    cosT = np.cos(ang).T.astype(np.float32)[pair_perm]                  # [HD, S]
    sinT = np.sin(ang).T.astype(np.float32)[pair_perm]
    sinT[0::2] *= -1.0
    cos_full = np.tile(cosT, (HL, 1)).astype(BF)
    sin_full = np.tile(sinT, (HL, 1)).astype(BF)

    # causal masks [4, 128, 512]: multiplicative (1 = keep, 0 = drop)
    a = np.arange(128)[:, None]
    b = np.arange(512)[None, :]
    maskT = np.stack([(a + 128 * i <= b) for i in range(4)]).astype(np.float32)
    maskT = maskT.astype(BF)

    in_maps = []
    for c in range(NC_CORES):
        er = slice(c * EL, (c + 1) * EL)
        fr = slice(c * FL, (c + 1) * FL)
        vr = slice(c * VL, (c + 1) * VL)
        dr = slice(c * 128, (c + 1) * 128)   # o_proj output d-shard
        wqT = np.stack([(Wq[l][er, :] * anw[l][None, :]).T[:, full_perm]
                        for l in range(L)])
        wkT = np.stack([(Wk[l][er, :] * anw[l][None, :]).T[:, full_perm]
                        for l in range(L)])
        wvT = np.stack([(Wv[l][er, :] * anw[l][None, :]).T for l in range(L)])
        woT = np.stack([np.ascontiguousarray(Wo[l][dr, :].T) for l in range(L)])
        wgT = np.stack([Wg[l][:, fr] * fnw[l][:, None] for l in range(L)])
        wuT = np.stack([Wu[l][:, fr] * fnw[l][:, None] for l in range(L)])
        wdT = np.stack([Wd[l][fr, :] for l in range(L)])
        embT = np.ascontiguousarray((emb[vr, :] * finw[None, :]).T)
        in_maps.append({
            "hid0T": hid0T,
            "wqT": wqT.astype(BF), "wkT": wkT.astype(BF), "wvT": wvT.astype(BF),
            "woT": woT.astype(BF), "wgT": wgT.astype(BF), "wuT": wuT.astype(BF),
            "wdT": wdT.astype(BF), "embT": embT.astype(BF),
            "cosT": cos_full, "sinT": sin_full, "maskT": maskT,
        })
    return in_maps


_RUNNER = None


def make_runner(nc):
    """Wrap a compiled Bacc module into a jitted 8-core callable."""
    import jax
    from jax.sharding import Mesh, PartitionSpec
    from jax.experimental.shard_map import shard_map
    from concourse.bass2jax import (_bass_exec_p, partition_id_tensor,
                                    install_neuronx_cc_hook)
    import jax.numpy as jnp

    install_neuronx_cc_hook()

    partition_name = nc.partition_id_tensor.name if nc.partition_id_tensor else None
    in_names, out_names, out_avals = [], [], []
    for alloc in nc.m.functions[0].allocations:
        if not isinstance(alloc, mybir.MemoryLocationSet):
            continue
        name = alloc.memorylocations[0].name
        if alloc.kind == "ExternalInput":
            if name != partition_name:
                in_names.append(name)
        elif alloc.kind == "ExternalOutput":
            out_names.append(name)
            out_avals.append(jax.core.ShapedArray(
                tuple(alloc.tensor_shape), mybir.dt.np(alloc.dtype)))
    n_params = len(in_names)
    in_names_all = list(in_names) + list(out_names)
    if partition_name is not None:
        in_names_all.append(partition_name)

    def _body(*args):
        operands = list(args)
        if partition_name is not None:
            operands.append(partition_id_tensor())
        outs = _bass_exec_p.bind(
            *operands,
            out_avals=tuple(out_avals),
            in_names=tuple(in_names_all),
            out_names=tuple(out_names),
            lowering_input_output_aliases=(),
            sim_require_finite=True,
            sim_require_nnan=True,
            nc=nc,
        )
        return tuple(outs)

    devices = jax.devices()[:NC_CORES]
    mesh = Mesh(np.asarray(devices), ("core",))
    n_outs = len(out_names)
    in_specs = (PartitionSpec("core"),) * (n_params + n_outs)
    out_specs = (PartitionSpec("core"),) * len(out_names)
    sharded = jax.jit(shard_map(_body, mesh=mesh, in_specs=in_specs,
                                out_specs=out_specs, check_rep=False),
                      keep_unused=True)

    def zero_outs():
        return [np.zeros((NC_CORES * av.shape[0], *av.shape[1:]), av.dtype)
                for av in out_avals]

    def run(in_maps):
        concat_in = [np.concatenate([np.asarray(in_maps[c][nm])
                                     for c in range(NC_CORES)], axis=0)
                     for nm in in_names]
        out_arrs = sharded(*concat_in, *zero_outs())
        import jax as _jax
        _jax.block_until_ready(out_arrs)
        return [
            {nm: np.asarray(out_arrs[i]).reshape(NC_CORES, *out_avals[i].shape)[c]
             for i, nm in enumerate(out_names)}
            for c in range(NC_CORES)
        ]

    run.zero_outs = zero_outs

    run.sharded = sharded
    run.in_names = in_names
    run.out_names = out_names
    run.out_avals = out_avals
    run.mesh = mesh
    run.nc = nc
    return run


def _get_runner():
    global _RUNNER
    if _RUNNER is None:
        _RUNNER = make_runner(build_nc())
    return _RUNNER


def kernel(**inputs) -> np.ndarray:
    in_maps = host_prep(inputs)
    run = _get_runner()
    results = run(in_maps)
    logits = np.concatenate([results[c]["logits"].astype(np.float32)
                             for c in range(NC_CORES)], axis=1)
    return logits.reshape(B, S, V)
